# revision 2
# baseline (speedup 1.0000x reference)
"""Banded (sliding-window) causal multi-head attention for Trainium2 — v3.

Problem: B=1, H=16, S=2048, DK=64 fp32; layer_idx=1 -> causal mask AND
(i - j) < 256 sliding window.  Returns (context, k, v) like the reference.

Sharding: 16 heads over 8 cores = 2 heads/core (pure head parallelism).

Pipeline (per head, 8 pair-cycles of 2 key blocks):
  PE:  st pair matmuls (K^T@Q scores) ... PV matmuls (P^T@[V|1]).
  ACT: exp over the [128, 2*384] score pair tile (PSUM fp32 -> SBUF fp16).
  DVE: ctx evacuation (PSUM fp32 -> fp16, incl denominators) and the edge
       sub-block masking.  The middle sub-block of every key block is fully
       inside the band, so its PV reads the exp output directly (no mask).
  Host: final ctx/denominator division + unshard.

Orderings that matter:
  - DVE program order: evac(ready now) BEFORE mask(p+1) (dep on exp(p+1)
    finishing about now) — avoids head-of-line blocking that stalls PV
    weight loads.
  - qt is loaded in 3 OVERLAPPING column chunks (each key block's 384-wide
    query span lives entirely in one chunk) so st matmuls never split.
  - Input DMAs ordered first-needed-first across the sync+scalar queues.
"""

import os
import sys

for _p in ("/opt/trn_rl_repo", os.path.expanduser("~/.axon_site/_ro/trn_rl_repo")):
    if os.path.isdir(_p) and _p not in sys.path:
        sys.path.insert(0, _p)

import numpy as np

B, H, S, DK = 1, 16, 2048, 64
LOCAL_WINDOW = 256
N_CORES = 8
HPC = H // N_CORES  # heads per core
TB = 128            # tile block
NKB = S // TB       # key blocks per head
G = 4               # query blocks per psum output group
VW = DK + 1         # V columns + ones column
SPAN = LOCAL_WINDOW + TB          # 384: query span per key block
NPAIR = NKB // 2    # 8

CH = 512  # non-overlapping qt/kt chunk width

_prog_cache = {}


def _build_banded():
    import concourse.bass as bass
    import concourse.tile as tile
    from concourse import bacc, mybir

    fp16 = mybir.dt.float16
    fp32 = mybir.dt.float32

    nc = bacc.Bacc("TRN2", target_bir_lowering=False, debug=False)
    qt_d = nc.dram_tensor("qt", [TB, S], fp16, kind="ExternalInput")
    kt_d = nc.dram_tensor("kt", [TB, S], fp16, kind="ExternalInput")
    va_d = nc.dram_tensor("va", [TB, HPC * NKB * VW], fp16, kind="ExternalInput")
    # mask: [TB, 256] = [sub0_mask | sub2_mask]
    mask_d = nc.dram_tensor("mask", [TB, 256], fp16, kind="ExternalInput")
    ctx_d = nc.dram_tensor("ctx", [HPC, 4, TB, G * VW], fp16,
                           kind="ExternalOutput")

    with tile.TileContext(nc) as tc:
        with (
            tc.tile_pool(name="inp", bufs=1) as inp,
            tc.tile_pool(name="exp", bufs=4) as expp,
            tc.tile_pool(name="pt", bufs=3) as ptp,
            tc.tile_pool(name="stp", bufs=3, space="PSUM") as stp,
            tc.tile_pool(name="ctxp", bufs=2, space="PSUM") as ctxp,
            tc.tile_pool(name="outp", bufs=3) as outp,
        ):
            qt_sb = [inp.tile([TB, CH], fp16, tag=f"qt{c}", name=f"qt_sb{c}")
                     for c in range(4)]
            kt_sb = [inp.tile([TB, CH], fp16, tag=f"kt{c}", name=f"kt_sb{c}")
                     for c in range(4)]
            va_cs = NKB * VW
            vh = va_cs // 2
            va_sb = [[inp.tile([TB, vh], fp16, tag=f"va{c}{q}",
                               name=f"va_sb{c}{q}") for q in range(2)]
                     for c in range(HPC)]
            mask_sb = inp.tile([TB, 256], fp16, tag="mask")

            # mask generated on the idle Pool engine: ones then band select
            nc.gpsimd.memset(mask_sb[:, 0:128], 1.0)
            nc.gpsimd.memset(mask_sb[:, 128:256], 1.0)
            # sub0: keep (=1) where q_loc - k_loc >= 0
            nc.gpsimd.affine_select(
                out=mask_sb[:, 0:128], in_=mask_sb[:, 0:128],
                compare_op=mybir.AluOpType.is_ge, fill=0.0,
                base=0, pattern=[[1, 128]], channel_multiplier=-1)
            # sub2: keep where k_loc - q_loc > 0
            nc.gpsimd.affine_select(
                out=mask_sb[:, 128:256], in_=mask_sb[:, 128:256],
                compare_op=mybir.AluOpType.is_gt, fill=0.0,
                base=0, pattern=[[-1, 128]], channel_multiplier=1)

            # tiny first tiles so pair 0 starts before the big chunks land
            kt00 = inp.tile([TB, TB], fp16, tag="kt00")
            qt00 = inp.tile([TB, SPAN], fp16, tag="qt00")
            nc.sync.dma_start(kt00[:], kt_d.ap()[:, 0:TB])
            nc.scalar.dma_start(qt00[:], qt_d.ap()[:, 0:SPAN])

            # scalar issues few DMAs (no sem-pool reuse waits that
            # would head-of-line block the ACT engine); the rest ride sync.
            # sync:   kt00 kt0 kt1 va0a kt2 kt3 va0b va1a va1b (+ outputs)
            # scalar: qt00 qt0 qt1 qt2 qt3
            nc.sync.dma_start(kt_sb[0][:], kt_d.ap()[:, 0:CH])
            nc.scalar.dma_start(qt_sb[0][:], qt_d.ap()[:, 0:CH])
            nc.sync.dma_start(kt_sb[1][:], kt_d.ap()[:, CH:2 * CH])
            nc.scalar.dma_start(qt_sb[1][:], qt_d.ap()[:, CH:2 * CH])
            nc.sync.dma_start(va_sb[0][0][:], va_d.ap()[:, 0:vh])
            nc.scalar.dma_start(qt_sb[2][:], qt_d.ap()[:, 2 * CH:3 * CH])
            nc.sync.dma_start(kt_sb[2][:], kt_d.ap()[:, 2 * CH:3 * CH])
            nc.scalar.dma_start(qt_sb[3][:], qt_d.ap()[:, 3 * CH:4 * CH])
            nc.sync.dma_start(kt_sb[3][:], kt_d.ap()[:, 3 * CH:4 * CH])
            nc.sync.dma_start(va_sb[0][1][:], va_d.ap()[:, vh:va_cs])
            nc.sync.dma_start(va_sb[1][0][:],
                              va_d.ap()[:, va_cs:va_cs + vh])
            nc.sync.dma_start(va_sb[1][1][:],
                              va_d.ap()[:, va_cs + vh:2 * va_cs])

            def kt_slice(kb, hr):
                lo = kb * TB
                c, off = divmod(lo, CH)
                return kt_sb[c][hr, off:off + TB]

            def qt_parts(kb, w):
                lo = kb * TB
                out, pos = [], lo
                while pos < lo + w:
                    c, off = divmod(pos, CH)
                    take = min(CH - off, lo + w - pos)
                    out.append((pos - lo, qt_sb[c], off, take))
                    pos += take
                # biggest part first: the run's final matmul is then short,
                # so the next PV's weight load overlaps less exposed stream
                out.sort(key=lambda t: -t[3])
                return out

            def va_slice(h, kb):
                t = va_sb[h][kb // 8]
                off = (kb % 8) * VW
                return t[:, off:off + VW]

            def kb_width(kb):
                return min(SPAN, S - kb * TB)

            for h in range(HPC):
                hr = slice(h * DK, (h + 1) * DK)
                ctx_tiles = {}
                started = set()
                e_tiles = {}
                pt_tiles = {}
                closed = []   # groups closed by PV, pending evac

                st_tiles = {}

                def emit_st_a(pair):
                    kb0 = 2 * pair
                    w0 = kb_width(kb0)
                    st = stp.tile([TB, 2 * 512], fp32, tag="st",
                                  name=f"st_{h}_{pair}")
                    st_tiles[pair] = st
                    if pair == 0:
                        # kb0: use the tiny early tiles
                        nc.tensor.matmul(st[:, 0:w0], lhsT=kt00[hr, :],
                                         rhs=qt00[hr, 0:w0],
                                         start=True, stop=True)
                        return
                    for co, qsb, off, take in qt_parts(kb0, w0):
                        nc.tensor.matmul(st[:, co:co + take],
                                         lhsT=kt_slice(kb0, hr),
                                         rhs=qsb[hr, off:off + take],
                                         start=True, stop=True)

                def emit_st_b(pair):
                    kb0 = 2 * pair
                    w1 = kb_width(kb0 + 1)
                    st = st_tiles[pair]
                    for co, qsb, off, take in qt_parts(kb0 + 1, w1):
                        nc.tensor.matmul(st[:, 512 + co:512 + co + take],
                                         lhsT=kt_slice(kb0 + 1, hr),
                                         rhs=qsb[hr, off:off + take],
                                         start=True, stop=True)

                def emit_exp(pair):
                    kb0 = 2 * pair
                    w0, w1 = kb_width(kb0), kb_width(kb0 + 1)
                    st = st_tiles.pop(pair)
                    e = expp.tile([TB, 2 * SPAN], fp16, tag="exp",
                                  name=f"e_{h}_{pair}")
                    if w0 == SPAN and w1 == SPAN:
                        st3 = st[:].rearrange("p (b c) -> p b c", c=512)[:, :, 0:SPAN]
                        e3 = e[:].rearrange("p (b c) -> p b c", c=SPAN)
                        nc.scalar.activation(
                            e3, st3, mybir.ActivationFunctionType.Exp)
                    else:
                        nc.scalar.activation(
                            e[:, 0:w0], st[:, 0:w0],
                            mybir.ActivationFunctionType.Exp)
                        if w1 > 0:
                            nc.scalar.activation(
                                e[:, SPAN:SPAN + w1], st[:, 512:512 + w1],
                                mybir.ActivationFunctionType.Exp)
                    e_tiles[pair] = e

                def emit_st_exp(pair):
                    emit_st_a(pair)
                    emit_st_b(pair)
                    emit_exp(pair)

                def emit_mask(pair):
                    # pt layout [TB, (half, b, 128)]: b=0 -> sub0 (causal
                    # triangle), b=1 -> sub2 (window triangle).
                    kb0 = 2 * pair
                    w0, w1 = kb_width(kb0), kb_width(kb0 + 1)
                    e = e_tiles[pair]
                    pt = ptp.tile([TB, 512], fp16, tag="pt",
                                  name=f"pt_{h}_{pair}")
                    if w0 == SPAN and w1 == SPAN:
                        ef = e[:].rearrange(
                            "p (hh b c) -> p hh b c", hh=2, c=128)  # [p,2,3,128]
                        pt4 = pt[:].rearrange(
                            "p (hh b c) -> p hh b c", hh=2, c=128)  # [p,2,2,128]
                        m0 = (mask_sb[:, 0:128].unsqueeze(1).unsqueeze(1)
                              .broadcast_to([TB, 2, 1, 128]))
                        m2 = (mask_sb[:, 128:256].unsqueeze(1).unsqueeze(1)
                              .broadcast_to([TB, 2, 1, 128]))
                        nc.vector.tensor_mul(
                            pt4[:, :, 0:1, :], ef[:, :, 0:1, :], m0)
                        nc.vector.tensor_mul(
                            pt4[:, :, 1:2, :], ef[:, :, 2:3, :], m2)
                    else:
                        # tail pair: kb14 has sub0+sub1 (w0=256), kb15 sub0
                        nc.vector.tensor_mul(
                            pt[:, 0:128], e[:, 0:128], mask_sb[:, 0:128])
                        if w1 > 0:
                            nc.vector.tensor_mul(
                                pt[:, 256:384], e[:, SPAN:SPAN + 128],
                                mask_sb[:, 0:128])
                    pt_tiles[pair] = pt

                def pv_one(kb, qb, lhsT):
                    g, j = divmod(qb, G)
                    if g not in ctx_tiles:
                        ctx_tiles[g] = ctxp.tile(
                            [TB, G * VW], fp32, tag="ctx",
                            name=f"ctx_{h}_{g}")
                    ct = ctx_tiles[g]
                    last = (qb == g * G + G - 1) and (kb == qb)
                    nc.tensor.matmul(
                        ct[:, j * VW:(j + 1) * VW], lhsT=lhsT,
                        rhs=va_slice(h, kb),
                        start=(g not in started), stop=last)
                    started.add(g)
                    if last:
                        closed.append(g)

                def emit_pv_batch(pair, batch):
                    kb0 = 2 * pair
                    e = e_tiles[pair]
                    pt = pt_tiles[pair]
                    if batch == 1:
                        # middles (dep: exp only) + first sub0 edge
                        for half, kb in ((0, kb0), (1, kb0 + 1)):
                            if kb + 1 <= NKB - 1 and kb_width(kb) >= 256:
                                pv_one(kb, kb + 1,
                                       e[:, half * SPAN + 128:half * SPAN + 256])
                        pv_one(kb0, kb0, pt[:, 0:128])
                    else:
                        if kb0 + 2 <= NKB - 1 and kb_width(kb0) == SPAN:
                            pv_one(kb0, kb0 + 2, pt[:, 128:256])
                        pv_one(kb0 + 1, kb0 + 1, pt[:, 256:384])
                        if kb0 + 3 <= NKB - 1 and kb_width(kb0 + 1) == SPAN:
                            pv_one(kb0 + 1, kb0 + 3, pt[:, 384:512])

                def emit_pv(pair):
                    emit_pv_batch(pair, 1)
                    emit_pv_batch(pair, 2)

                def emit_evacs():
                    while closed:
                        g = closed.pop(0)
                        out_sb = outp.tile([TB, G * VW], fp16, tag="out",
                                           name=f"out_{h}_{g}")
                        nc.vector.tensor_scalar_mul(
                            out_sb[:], ctx_tiles[g][:], 1.0)
                        nc.sync.dma_start(ctx_d.ap()[h, g], out_sb[:])
                        del ctx_tiles[g]
                        started.discard(g)

                # software pipeline, 2-pair supercycles: long st runs and
                # long PV runs minimize PE weight-load exposure at run
                # boundaries.
                emit_st_exp(0)
                emit_st_exp(1)
                emit_mask(0)
                for c in range(NPAIR // 2):
                    p0, p1 = 2 * c, 2 * c + 1
                    n0, n1 = 2 * c + 2, 2 * c + 3
                    if n0 < NPAIR:
                        emit_st_a(n0)
                        emit_st_b(n0)
                    if n1 < NPAIR:
                        emit_st_a(n1)
                        emit_st_b(n1)
                    emit_evacs()
                    if n0 < NPAIR:
                        emit_exp(n0)
                    if n1 < NPAIR:
                        emit_exp(n1)
                    if p0 + 1 < NPAIR:
                        emit_mask(p0 + 1)
                    if p0 + 2 < NPAIR:
                        emit_mask(p0 + 2)
                    emit_pv(p0)
                    emit_pv(p1)
                emit_evacs()
    nc.finalize()
    return nc


def _build_causal():
    """Correctness fallback for even layer_idx (full causal attention)."""
    import concourse.bass as bass
    import concourse.tile as tile
    from concourse import bacc, mybir

    fp16 = mybir.dt.float16
    fp32 = mybir.dt.float32
    mwidth = 512

    nc = bacc.Bacc("TRN2", target_bir_lowering=False, debug=False)
    qt_d = nc.dram_tensor("qt", [TB, S], fp16, kind="ExternalInput")
    kt_d = nc.dram_tensor("kt", [TB, S], fp16, kind="ExternalInput")
    va_d = nc.dram_tensor("va", [TB, HPC * NKB * VW], fp16, kind="ExternalInput")
    mask_d = nc.dram_tensor("mask", [TB, mwidth], fp16, kind="ExternalInput")
    ctx_d = nc.dram_tensor("ctx", [HPC, 4, TB, G * VW], fp16,
                           kind="ExternalOutput")

    with tile.TileContext(nc) as tc:
        with (
            tc.tile_pool(name="inp", bufs=1) as inp,
            tc.tile_pool(name="exp", bufs=3) as expp,
            tc.tile_pool(name="pt", bufs=4) as ptp,
            tc.tile_pool(name="stp", bufs=2, space="PSUM") as stp,
            tc.tile_pool(name="ctxp", bufs=4, space="PSUM") as ctxp,
            tc.tile_pool(name="outp", bufs=3) as outp,
        ):
            mask_sb = inp.tile([TB, mwidth], fp16, tag="mask")
            nc.sync.dma_start(mask_sb[:], mask_d.ap())
            qt_sb = inp.tile([TB, S], fp16, tag="qt")
            nc.sync.dma_start(qt_sb[:], qt_d.ap())
            kt_sb = inp.tile([TB, S], fp16, tag="kt")
            nc.sync.dma_start(kt_sb[:], kt_d.ap())
            va_sb = inp.tile([TB, HPC * NKB * VW], fp16, tag="va")
            nc.sync.dma_start(va_sb[:], va_d.ap())

            for h in range(HPC):
                hr = slice(h * DK, (h + 1) * DK)
                ctx_tiles = {}
                started = set()
                for kb in range(NKB):
                    span = S - kb * TB
                    chunks = []
                    for o in range(0, span, 512):
                        w = min(512, span - o)
                        st = stp.tile([TB, 512], fp32, tag="st",
                                      name=f"st_{h}_{kb}_{o}")
                        nc.tensor.matmul(
                            st[:, 0:w], lhsT=kt_sb[hr, kb * TB:kb * TB + TB],
                            rhs=qt_sb[hr, kb * TB + o:kb * TB + o + w],
                            start=True, stop=True)
                        pt = ptp.tile([TB, 512], fp16, tag="pt",
                                      name=f"pt_{h}_{kb}_{o}")
                        if o == 0:
                            e = expp.tile([TB, 512], fp16, tag="exp",
                                          name=f"e_{h}_{kb}_{o}")
                            nc.scalar.activation(
                                e[:, 0:w], st[:, 0:w],
                                mybir.ActivationFunctionType.Exp)
                            nc.vector.tensor_mul(
                                pt[:, 0:w], e[:, 0:w], mask_sb[:, 0:w])
                        else:
                            nc.scalar.activation(
                                pt[:, 0:w], st[:, 0:w],
                                mybir.ActivationFunctionType.Exp)
                        chunks.append(pt)

                    for qb in range(kb, NKB):
                        g, j = divmod(qb, G)
                        if g not in ctx_tiles:
                            ctx_tiles[g] = ctxp.tile(
                                [TB, G * VW], fp32, tag="ctx", name=f"ctx_{h}_{g}")
                        ct = ctx_tiles[g]
                        o = (qb - kb) * TB
                        src = chunks[o // 512]
                        oo = o % 512
                        last = (qb == g * G + G - 1) and (kb == qb)
                        nc.tensor.matmul(
                            ct[:, j * VW:(j + 1) * VW],
                            lhsT=src[:, oo:oo + TB],
                            rhs=va_sb[:, (h * NKB + kb) * VW:(h * NKB + kb + 1) * VW],
                            start=(g not in started), stop=last)
                        started.add(g)
                        if last:
                            out_sb = outp.tile([TB, G * VW], fp16, tag="out",
                                               name=f"out_{h}_{g}")
                            nc.vector.tensor_scalar_mul(out_sb[:], ct[:], 1.0)
                            nc.sync.dma_start(ctx_d.ap()[h, g], out_sb[:])
                            del ctx_tiles[g]
                            started.discard(g)
    nc.finalize()
    return nc


def _get_program(win):
    if win not in _prog_cache:
        _prog_cache[win] = (
            _build_banded() if win == LOCAL_WINDOW else _build_causal())
    return _prog_cache[win]


def _make_mask_np(win):
    kl = np.arange(TB)[:, None]
    if win == LOCAL_WINDOW:
        c = np.arange(128)[None, :]
        m = np.zeros((TB, 256), np.float16)
        m[:, 0:128] = c >= kl       # sub0 causal: q_local >= k_local
        m[:, 128:256] = c < kl      # sub2 window: q_local < k_local
        return m
    qs = np.arange(512)[None, :]
    return ((qs - kl) >= 0).astype(np.float16)


def make_in_maps(q, k, v, win):
    scale = np.float32(1.0 / np.sqrt(DK))
    mask_np = _make_mask_np(win)
    in_maps = []
    for c in range(N_CORES):
        heads = range(c * HPC, (c + 1) * HPC)
        qt = np.concatenate(
            [(q[0, h] * scale).T for h in heads], axis=0).astype(np.float16)
        kt = np.concatenate(
            [k[0, h].T for h in heads], axis=0).astype(np.float16)
        va = np.empty((TB, HPC * NKB * VW), np.float16)
        for hi, h in enumerate(heads):
            vh = np.concatenate(
                [v[0, h], np.ones((S, 1), np.float32)], axis=1)  # [S, 65]
            va[:, hi * NKB * VW:(hi + 1) * NKB * VW] = (
                vh.reshape(NKB, TB, VW).transpose(1, 0, 2).reshape(TB, NKB * VW)
            ).astype(np.float16)
        in_maps.append({
            "qt": np.ascontiguousarray(qt),
            "kt": np.ascontiguousarray(kt),
            "va": np.ascontiguousarray(va),
            "mask": mask_np,
        })
    return in_maps


def _unshard(res):
    """res.results[c]["ctx"]: [HPC, 4, TB, G*VW] fp16 raw (ctx | denom)."""
    ctx = np.empty((B, H, S, DK), np.float32)
    for c in range(N_CORES):
        out = np.asarray(res.results[c]["ctx"], dtype=np.float32)
        for hi in range(HPC):
            t = out[hi].reshape(4, TB, G, VW)          # [g, p, n, VW]
            raw = t[:, :, :, 0:DK]                      # [g, p, n, 64]
            den = t[:, :, :, DK]                        # [g, p, n]
            full = raw / den[..., None]
            # q = g*512 + n*128 + p
            full = full.transpose(0, 2, 1, 3).reshape(S, DK)
            ctx[0, c * HPC + hi] = full
    return ctx


def kernel(q, k, v, layer_idx=1, training=0):
    from concourse.bass_utils import run_bass_kernel_spmd

    q = np.asarray(q)
    k = np.asarray(k)
    v = np.asarray(v)
    li = int(np.asarray(layer_idx))
    win = S if li % 2 == 0 else LOCAL_WINDOW

    nc = _get_program(win)
    in_maps = make_in_maps(q, k, v, win)
    res = run_bass_kernel_spmd(nc, in_maps, core_ids=list(range(N_CORES)))
    return _unshard(res), k, v


# revision 3
# speedup vs baseline: 1.0253x; 1.0253x over previous
"""Banded (sliding-window) causal multi-head attention for Trainium2 — v3.

Problem: B=1, H=16, S=2048, DK=64 fp32; layer_idx=1 -> causal mask AND
(i - j) < 256 sliding window.  Returns (context, k, v) like the reference.

Sharding: 16 heads over 8 cores = 2 heads/core (pure head parallelism).

Pipeline (per head, 8 pair-cycles of 2 key blocks):
  PE:  st pair matmuls (K^T@Q scores) ... PV matmuls (P^T@[V|1]).
  ACT: exp over the [128, 2*384] score pair tile (PSUM fp32 -> SBUF fp16).
  DVE: ctx evacuation (PSUM fp32 -> fp16, incl denominators) and the edge
       sub-block masking.  The middle sub-block of every key block is fully
       inside the band, so its PV reads the exp output directly (no mask).
  Host: final ctx/denominator division + unshard.

Orderings that matter:
  - DVE program order: evac(ready now) BEFORE mask(p+1) (dep on exp(p+1)
    finishing about now) — avoids head-of-line blocking that stalls PV
    weight loads.
  - qt is loaded in 3 OVERLAPPING column chunks (each key block's 384-wide
    query span lives entirely in one chunk) so st matmuls never split.
  - Input DMAs ordered first-needed-first across the sync+scalar queues.
"""

import os
import sys

for _p in ("/opt/trn_rl_repo", os.path.expanduser("~/.axon_site/_ro/trn_rl_repo")):
    if os.path.isdir(_p) and _p not in sys.path:
        sys.path.insert(0, _p)

import numpy as np

B, H, S, DK = 1, 16, 2048, 64
LOCAL_WINDOW = 256
N_CORES = 8
HPC = H // N_CORES  # heads per core
TB = 128            # tile block
NKB = S // TB       # key blocks per head
G = 4               # query blocks per psum output group
VW = DK + 1         # V columns + ones column
SPAN = LOCAL_WINDOW + TB          # 384: query span per key block
NPAIR = NKB // 2    # 8

CH = 512  # non-overlapping qt/kt chunk width

_prog_cache = {}


def _build_banded():
    import concourse.bass as bass
    import concourse.tile as tile
    from concourse import bacc, mybir

    fp16 = mybir.dt.float16
    fp32 = mybir.dt.float32

    nc = bacc.Bacc("TRN2", target_bir_lowering=False, debug=False)
    qt_d = nc.dram_tensor("qt", [TB, S], fp16, kind="ExternalInput")
    kt_d = nc.dram_tensor("kt", [TB, S], fp16, kind="ExternalInput")
    va_d = nc.dram_tensor("va", [TB, HPC * NKB * VW], fp16, kind="ExternalInput")
    # mask: [TB, 256] = [sub0_mask | sub2_mask]
    mask_d = nc.dram_tensor("mask", [TB, 256], fp16, kind="ExternalInput")
    ctx_d = nc.dram_tensor("ctx", [HPC, 4, TB, G * VW], fp16,
                           kind="ExternalOutput")

    with tile.TileContext(nc) as tc:
        with (
            tc.tile_pool(name="inp", bufs=1) as inp,
            tc.tile_pool(name="exp", bufs=6) as expp,
            tc.tile_pool(name="pt", bufs=5) as ptp,
            tc.tile_pool(name="stp", bufs=3, space="PSUM") as stp,
            tc.tile_pool(name="ctxp", bufs=2, space="PSUM") as ctxp,
            tc.tile_pool(name="outp", bufs=4) as outp,
        ):
            qt_sb = [inp.tile([TB, CH], fp16, tag=f"qt{c}", name=f"qt_sb{c}")
                     for c in range(4)]
            kt_sb = [inp.tile([TB, CH], fp16, tag=f"kt{c}", name=f"kt_sb{c}")
                     for c in range(4)]
            va_cs = NKB * VW
            vh = va_cs // 2
            va_sb = [[inp.tile([TB, vh], fp16, tag=f"va{c}{q}",
                               name=f"va_sb{c}{q}") for q in range(2)]
                     for c in range(HPC)]
            mask_sb = inp.tile([TB, 256], fp16, tag="mask")

            # mask generated on the idle Pool engine: ones then band select
            nc.gpsimd.memset(mask_sb[:, 0:128], 1.0)
            nc.gpsimd.memset(mask_sb[:, 128:256], 1.0)
            # sub0: keep (=1) where q_loc - k_loc >= 0
            nc.gpsimd.affine_select(
                out=mask_sb[:, 0:128], in_=mask_sb[:, 0:128],
                compare_op=mybir.AluOpType.is_ge, fill=0.0,
                base=0, pattern=[[1, 128]], channel_multiplier=-1)
            # sub2: keep where k_loc - q_loc > 0
            nc.gpsimd.affine_select(
                out=mask_sb[:, 128:256], in_=mask_sb[:, 128:256],
                compare_op=mybir.AluOpType.is_gt, fill=0.0,
                base=0, pattern=[[-1, 128]], channel_multiplier=1)

            # tiny first tiles so pair 0 starts before the big chunks land
            kt00 = inp.tile([TB, TB], fp16, tag="kt00")
            qt00 = inp.tile([TB, SPAN], fp16, tag="qt00")
            nc.sync.dma_start(kt00[:], kt_d.ap()[:, 0:TB])
            nc.scalar.dma_start(qt00[:], qt_d.ap()[:, 0:SPAN])

            # scalar issues few DMAs (no sem-pool reuse waits that
            # would head-of-line block the ACT engine); the rest ride sync.
            # sync:   kt00 kt0 kt1 va0a kt2 kt3 va0b va1a va1b (+ outputs)
            # scalar: qt00 qt0 qt1 qt2 qt3
            nc.sync.dma_start(kt_sb[0][:], kt_d.ap()[:, 0:CH])
            nc.scalar.dma_start(qt_sb[0][:], qt_d.ap()[:, 0:CH])
            nc.sync.dma_start(kt_sb[1][:], kt_d.ap()[:, CH:2 * CH])
            nc.scalar.dma_start(qt_sb[1][:], qt_d.ap()[:, CH:2 * CH])
            nc.sync.dma_start(va_sb[0][0][:], va_d.ap()[:, 0:vh])
            nc.scalar.dma_start(qt_sb[2][:], qt_d.ap()[:, 2 * CH:3 * CH])
            nc.sync.dma_start(kt_sb[2][:], kt_d.ap()[:, 2 * CH:3 * CH])
            nc.scalar.dma_start(qt_sb[3][:], qt_d.ap()[:, 3 * CH:4 * CH])
            nc.sync.dma_start(kt_sb[3][:], kt_d.ap()[:, 3 * CH:4 * CH])
            nc.sync.dma_start(va_sb[0][1][:], va_d.ap()[:, vh:va_cs])
            nc.sync.dma_start(va_sb[1][0][:],
                              va_d.ap()[:, va_cs:va_cs + vh])
            nc.sync.dma_start(va_sb[1][1][:],
                              va_d.ap()[:, va_cs + vh:2 * va_cs])

            def kt_slice(kb, hr):
                lo = kb * TB
                c, off = divmod(lo, CH)
                return kt_sb[c][hr, off:off + TB]

            def qt_parts(kb, w):
                lo = kb * TB
                out, pos = [], lo
                while pos < lo + w:
                    c, off = divmod(pos, CH)
                    take = min(CH - off, lo + w - pos)
                    out.append((pos - lo, qt_sb[c], off, take))
                    pos += take
                # biggest part first: the run's final matmul is then short,
                # so the next PV's weight load overlaps less exposed stream
                out.sort(key=lambda t: -t[3])
                return out

            def va_slice(h, kb):
                t = va_sb[h][kb // 8]
                off = (kb % 8) * VW
                return t[:, off:off + VW]

            def kb_width(kb):
                return min(SPAN, S - kb * TB)

            for h in range(HPC):
                hr = slice(h * DK, (h + 1) * DK)
                ctx_tiles = {}
                started = set()
                e_tiles = {}
                pt_tiles = {}
                closed = []   # groups closed by PV, pending evac

                st_tiles = {}

                def emit_st_a(pair):
                    kb0 = 2 * pair
                    w0 = kb_width(kb0)
                    st = stp.tile([TB, 2 * 512], fp32, tag="st",
                                  name=f"st_{h}_{pair}")
                    st_tiles[pair] = st
                    if pair == 0:
                        # kb0: use the tiny early tiles
                        nc.tensor.matmul(st[:, 0:w0], lhsT=kt00[hr, :],
                                         rhs=qt00[hr, 0:w0],
                                         start=True, stop=True)
                        return
                    for co, qsb, off, take in qt_parts(kb0, w0):
                        nc.tensor.matmul(st[:, co:co + take],
                                         lhsT=kt_slice(kb0, hr),
                                         rhs=qsb[hr, off:off + take],
                                         start=True, stop=True)

                def emit_st_b(pair):
                    kb0 = 2 * pair
                    w1 = kb_width(kb0 + 1)
                    st = st_tiles[pair]
                    for co, qsb, off, take in qt_parts(kb0 + 1, w1):
                        nc.tensor.matmul(st[:, 512 + co:512 + co + take],
                                         lhsT=kt_slice(kb0 + 1, hr),
                                         rhs=qsb[hr, off:off + take],
                                         start=True, stop=True)

                def emit_exp(pair):
                    kb0 = 2 * pair
                    w0, w1 = kb_width(kb0), kb_width(kb0 + 1)
                    st = st_tiles.pop(pair)
                    e = expp.tile([TB, 2 * SPAN], fp16, tag="exp",
                                  name=f"e_{h}_{pair}")
                    if w0 == SPAN and w1 == SPAN:
                        st3 = st[:].rearrange("p (b c) -> p b c", c=512)[:, :, 0:SPAN]
                        e3 = e[:].rearrange("p (b c) -> p b c", c=SPAN)
                        nc.scalar.activation(
                            e3, st3, mybir.ActivationFunctionType.Exp)
                    else:
                        nc.scalar.activation(
                            e[:, 0:w0], st[:, 0:w0],
                            mybir.ActivationFunctionType.Exp)
                        if w1 > 0:
                            nc.scalar.activation(
                                e[:, SPAN:SPAN + w1], st[:, 512:512 + w1],
                                mybir.ActivationFunctionType.Exp)
                    e_tiles[pair] = e

                def emit_st_exp(pair):
                    emit_st_a(pair)
                    emit_st_b(pair)
                    emit_exp(pair)

                def emit_mask(pair):
                    # pt layout [TB, (half, b, 128)]: b=0 -> sub0 (causal
                    # triangle), b=1 -> sub2 (window triangle).
                    kb0 = 2 * pair
                    w0, w1 = kb_width(kb0), kb_width(kb0 + 1)
                    e = e_tiles[pair]
                    pt = ptp.tile([TB, 512], fp16, tag="pt",
                                  name=f"pt_{h}_{pair}")
                    if w0 == SPAN and w1 == SPAN:
                        ef = e[:].rearrange(
                            "p (hh b c) -> p hh b c", hh=2, c=128)  # [p,2,3,128]
                        pt4 = pt[:].rearrange(
                            "p (hh b c) -> p hh b c", hh=2, c=128)  # [p,2,2,128]
                        m0 = (mask_sb[:, 0:128].unsqueeze(1).unsqueeze(1)
                              .broadcast_to([TB, 2, 1, 128]))
                        m2 = (mask_sb[:, 128:256].unsqueeze(1).unsqueeze(1)
                              .broadcast_to([TB, 2, 1, 128]))
                        nc.vector.tensor_mul(
                            pt4[:, :, 0:1, :], ef[:, :, 0:1, :], m0)
                        nc.vector.tensor_mul(
                            pt4[:, :, 1:2, :], ef[:, :, 2:3, :], m2)
                    else:
                        # tail pair: kb14 has sub0+sub1 (w0=256), kb15 sub0
                        nc.vector.tensor_mul(
                            pt[:, 0:128], e[:, 0:128], mask_sb[:, 0:128])
                        if w1 > 0:
                            nc.vector.tensor_mul(
                                pt[:, 256:384], e[:, SPAN:SPAN + 128],
                                mask_sb[:, 0:128])
                    pt_tiles[pair] = pt

                def pv_one(kb, qb, lhsT):
                    g, j = divmod(qb, G)
                    if g not in ctx_tiles:
                        ctx_tiles[g] = ctxp.tile(
                            [TB, G * VW], fp32, tag="ctx",
                            name=f"ctx_{h}_{g}")
                    ct = ctx_tiles[g]
                    last = (qb == g * G + G - 1) and (kb == qb)
                    nc.tensor.matmul(
                        ct[:, j * VW:(j + 1) * VW], lhsT=lhsT,
                        rhs=va_slice(h, kb),
                        start=(g not in started), stop=last)
                    started.add(g)
                    if last:
                        closed.append(g)

                def emit_pv_batch(pair, batch):
                    kb0 = 2 * pair
                    e = e_tiles[pair]
                    pt = pt_tiles[pair]
                    if batch == 1:
                        # middles (dep: exp only) + first sub0 edge
                        for half, kb in ((0, kb0), (1, kb0 + 1)):
                            if kb + 1 <= NKB - 1 and kb_width(kb) >= 256:
                                pv_one(kb, kb + 1,
                                       e[:, half * SPAN + 128:half * SPAN + 256])
                        pv_one(kb0, kb0, pt[:, 0:128])
                    else:
                        if kb0 + 2 <= NKB - 1 and kb_width(kb0) == SPAN:
                            pv_one(kb0, kb0 + 2, pt[:, 128:256])
                        pv_one(kb0 + 1, kb0 + 1, pt[:, 256:384])
                        if kb0 + 3 <= NKB - 1 and kb_width(kb0 + 1) == SPAN:
                            pv_one(kb0 + 1, kb0 + 3, pt[:, 384:512])

                def emit_pv(pair):
                    emit_pv_batch(pair, 1)
                    emit_pv_batch(pair, 2)

                def emit_evacs():
                    while closed:
                        g = closed.pop(0)
                        out_sb = outp.tile([TB, G * VW], fp16, tag="out",
                                           name=f"out_{h}_{g}")
                        nc.vector.tensor_scalar_mul(
                            out_sb[:], ctx_tiles[g][:], 1.0)
                        nc.sync.dma_start(ctx_d.ap()[h, g], out_sb[:])
                        del ctx_tiles[g]
                        started.discard(g)

                # software pipeline, 2-pair supercycles: long st runs and
                # long PV runs minimize PE weight-load exposure at run
                # boundaries.
                emit_st_exp(0)
                emit_st_exp(1)
                emit_mask(0)
                for c in range(NPAIR // 2):
                    p0, p1 = 2 * c, 2 * c + 1
                    n0, n1 = 2 * c + 2, 2 * c + 3
                    if n0 < NPAIR:
                        emit_st_a(n0)
                        emit_st_b(n0)
                    if n1 < NPAIR:
                        emit_st_a(n1)
                        emit_st_b(n1)
                    emit_evacs()
                    if n0 < NPAIR:
                        emit_exp(n0)
                    if n1 < NPAIR:
                        emit_exp(n1)
                    if p0 + 1 < NPAIR:
                        emit_mask(p0 + 1)
                    if p0 + 2 < NPAIR:
                        emit_mask(p0 + 2)
                    emit_pv(p0)
                    emit_pv(p1)
                emit_evacs()
    nc.finalize()
    return nc


def _build_causal():
    """Correctness fallback for even layer_idx (full causal attention)."""
    import concourse.bass as bass
    import concourse.tile as tile
    from concourse import bacc, mybir

    fp16 = mybir.dt.float16
    fp32 = mybir.dt.float32
    mwidth = 512

    nc = bacc.Bacc("TRN2", target_bir_lowering=False, debug=False)
    qt_d = nc.dram_tensor("qt", [TB, S], fp16, kind="ExternalInput")
    kt_d = nc.dram_tensor("kt", [TB, S], fp16, kind="ExternalInput")
    va_d = nc.dram_tensor("va", [TB, HPC * NKB * VW], fp16, kind="ExternalInput")
    mask_d = nc.dram_tensor("mask", [TB, mwidth], fp16, kind="ExternalInput")
    ctx_d = nc.dram_tensor("ctx", [HPC, 4, TB, G * VW], fp16,
                           kind="ExternalOutput")

    with tile.TileContext(nc) as tc:
        with (
            tc.tile_pool(name="inp", bufs=1) as inp,
            tc.tile_pool(name="exp", bufs=3) as expp,
            tc.tile_pool(name="pt", bufs=4) as ptp,
            tc.tile_pool(name="stp", bufs=2, space="PSUM") as stp,
            tc.tile_pool(name="ctxp", bufs=4, space="PSUM") as ctxp,
            tc.tile_pool(name="outp", bufs=3) as outp,
        ):
            mask_sb = inp.tile([TB, mwidth], fp16, tag="mask")
            nc.sync.dma_start(mask_sb[:], mask_d.ap())
            qt_sb = inp.tile([TB, S], fp16, tag="qt")
            nc.sync.dma_start(qt_sb[:], qt_d.ap())
            kt_sb = inp.tile([TB, S], fp16, tag="kt")
            nc.sync.dma_start(kt_sb[:], kt_d.ap())
            va_sb = inp.tile([TB, HPC * NKB * VW], fp16, tag="va")
            nc.sync.dma_start(va_sb[:], va_d.ap())

            for h in range(HPC):
                hr = slice(h * DK, (h + 1) * DK)
                ctx_tiles = {}
                started = set()
                for kb in range(NKB):
                    span = S - kb * TB
                    chunks = []
                    for o in range(0, span, 512):
                        w = min(512, span - o)
                        st = stp.tile([TB, 512], fp32, tag="st",
                                      name=f"st_{h}_{kb}_{o}")
                        nc.tensor.matmul(
                            st[:, 0:w], lhsT=kt_sb[hr, kb * TB:kb * TB + TB],
                            rhs=qt_sb[hr, kb * TB + o:kb * TB + o + w],
                            start=True, stop=True)
                        pt = ptp.tile([TB, 512], fp16, tag="pt",
                                      name=f"pt_{h}_{kb}_{o}")
                        if o == 0:
                            e = expp.tile([TB, 512], fp16, tag="exp",
                                          name=f"e_{h}_{kb}_{o}")
                            nc.scalar.activation(
                                e[:, 0:w], st[:, 0:w],
                                mybir.ActivationFunctionType.Exp)
                            nc.vector.tensor_mul(
                                pt[:, 0:w], e[:, 0:w], mask_sb[:, 0:w])
                        else:
                            nc.scalar.activation(
                                pt[:, 0:w], st[:, 0:w],
                                mybir.ActivationFunctionType.Exp)
                        chunks.append(pt)

                    for qb in range(kb, NKB):
                        g, j = divmod(qb, G)
                        if g not in ctx_tiles:
                            ctx_tiles[g] = ctxp.tile(
                                [TB, G * VW], fp32, tag="ctx", name=f"ctx_{h}_{g}")
                        ct = ctx_tiles[g]
                        o = (qb - kb) * TB
                        src = chunks[o // 512]
                        oo = o % 512
                        last = (qb == g * G + G - 1) and (kb == qb)
                        nc.tensor.matmul(
                            ct[:, j * VW:(j + 1) * VW],
                            lhsT=src[:, oo:oo + TB],
                            rhs=va_sb[:, (h * NKB + kb) * VW:(h * NKB + kb + 1) * VW],
                            start=(g not in started), stop=last)
                        started.add(g)
                        if last:
                            out_sb = outp.tile([TB, G * VW], fp16, tag="out",
                                               name=f"out_{h}_{g}")
                            nc.vector.tensor_scalar_mul(out_sb[:], ct[:], 1.0)
                            nc.sync.dma_start(ctx_d.ap()[h, g], out_sb[:])
                            del ctx_tiles[g]
                            started.discard(g)
    nc.finalize()
    return nc


def _get_program(win):
    if win not in _prog_cache:
        _prog_cache[win] = (
            _build_banded() if win == LOCAL_WINDOW else _build_causal())
    return _prog_cache[win]


def _make_mask_np(win):
    kl = np.arange(TB)[:, None]
    if win == LOCAL_WINDOW:
        c = np.arange(128)[None, :]
        m = np.zeros((TB, 256), np.float16)
        m[:, 0:128] = c >= kl       # sub0 causal: q_local >= k_local
        m[:, 128:256] = c < kl      # sub2 window: q_local < k_local
        return m
    qs = np.arange(512)[None, :]
    return ((qs - kl) >= 0).astype(np.float16)


def make_in_maps(q, k, v, win):
    scale = np.float32(1.0 / np.sqrt(DK))
    mask_np = _make_mask_np(win)
    in_maps = []
    for c in range(N_CORES):
        heads = range(c * HPC, (c + 1) * HPC)
        qt = np.concatenate(
            [(q[0, h] * scale).T for h in heads], axis=0).astype(np.float16)
        kt = np.concatenate(
            [k[0, h].T for h in heads], axis=0).astype(np.float16)
        va = np.empty((TB, HPC * NKB * VW), np.float16)
        for hi, h in enumerate(heads):
            vh = np.concatenate(
                [v[0, h], np.ones((S, 1), np.float32)], axis=1)  # [S, 65]
            va[:, hi * NKB * VW:(hi + 1) * NKB * VW] = (
                vh.reshape(NKB, TB, VW).transpose(1, 0, 2).reshape(TB, NKB * VW)
            ).astype(np.float16)
        in_maps.append({
            "qt": np.ascontiguousarray(qt),
            "kt": np.ascontiguousarray(kt),
            "va": np.ascontiguousarray(va),
            "mask": mask_np,
        })
    return in_maps


def _unshard(res):
    """res.results[c]["ctx"]: [HPC, 4, TB, G*VW] fp16 raw (ctx | denom)."""
    ctx = np.empty((B, H, S, DK), np.float32)
    for c in range(N_CORES):
        out = np.asarray(res.results[c]["ctx"], dtype=np.float32)
        for hi in range(HPC):
            t = out[hi].reshape(4, TB, G, VW)          # [g, p, n, VW]
            raw = t[:, :, :, 0:DK]                      # [g, p, n, 64]
            den = t[:, :, :, DK]                        # [g, p, n]
            full = raw / den[..., None]
            # q = g*512 + n*128 + p
            full = full.transpose(0, 2, 1, 3).reshape(S, DK)
            ctx[0, c * HPC + hi] = full
    return ctx


def kernel(q, k, v, layer_idx=1, training=0):
    from concourse.bass_utils import run_bass_kernel_spmd

    q = np.asarray(q)
    k = np.asarray(k)
    v = np.asarray(v)
    li = int(np.asarray(layer_idx))
    win = S if li % 2 == 0 else LOCAL_WINDOW

    nc = _get_program(win)
    in_maps = make_in_maps(q, k, v, win)
    res = run_bass_kernel_spmd(nc, in_maps, core_ids=list(range(N_CORES)))
    return _unshard(res), k, v
